# revision 3
# baseline (speedup 1.0000x reference)
"""Trainium2 Bass kernel for nn_CombinedGNN (gnn_message_passing) — v7.

8 NeuronCores, node/row parallel, zero collectives.
  - masks[1] is identically zero; only mask0 = adj/rowdeg matters. All T=12
    aggregations batch into one matmul agg^T = X^T @ adjT per core.
  - adjacency ships as raw 0/1 fp8 (exact); host-computed 1/deg is applied
    on-chip by one DVE multiply per node-half (rdeg broadcast shipped
    pre-replicated, bf16).
  - adjacency DMA: single SP HWDGE ring, full-width k-chunks with 6.4KB
    contiguous per-partition runs (~400GB/s measured); phase-1 matmuls
    chase the chunks, halves interleaved per k-tile.
  - chain uses the dag[24, T, NP] layout (prev | raw | agg stacked) so each
    t-step is ONE p1 matmul; agg slabs are scattered into dag by SBUF->SBUF
    DMAs issued from the idle Sync engine, pipelined ahead of the chain.
  - warmup + filler matmuls keep the PE clock (HAM) at full rate.
"""

import numpy as np
import ml_dtypes

import concourse.bass as bass
import concourse.mybir as mybir
import concourse.bass_utils as bass_utils
from concourse.tile import TileContext
from concourse.vector_clock import ScopedClock
from contextlib import contextmanager


@contextmanager
def _lean_drain():
    """Skip end-of-kernel semaphore clears (one-shot NEFF; every
    run_bass_kernel_spmd call reloads the NEFF, which re-zeros sems)."""
    orig = TileContext._drain_and_barrier

    def patched(self, tick_clock, wait_clock):
        nc = self.nc
        drain_inst = nc.sync.drain()
        wait_clock.add_sem_waits(
            drain_inst.ins, ScopedClock({None: tick_clock.global_clock}))
        nc.all_engine_barrier()
        popped = nc._tile_sem_poison_stack.pop()
        assert popped is self._sem_poison
        nc.all_engine_barrier()

    TileContext._drain_and_barrier = patched
    try:
        yield
    finally:
        TileContext._drain_and_barrier = orig


# problem constants (hardcoded per harness contract)
N, T, DAY, L = 5000, 12, 8, 2
F = DAY - 1
DIM = T * DAY  # 96
NCORES = 8
NPC = N // NCORES        # 625 nodes per core
NP = 640                 # padded nodes per core
NH = NP // 2             # 320, node half per psum tile
KT = 128                 # contraction tile (partitions)
NK = 5120                # padded contraction size
NKT = NK // KT           # 40
KG = 5                   # k-tiles per DMA chunk
NG = NKT // KG           # 4 chunks
NWU = 24                 # warmup matmuls

F32 = mybir.dt.float32
BF16 = mybir.dt.bfloat16
FP8 = mybir.dt.float8e4
BF16_NP = ml_dtypes.bfloat16
FP8_NP = ml_dtypes.float8_e4m3fn

_MAXW = 1


def split_multi_waits(nc):
    """Walrus in this container rejects instructions with >~2 sync waits.
    Hoist extra waits onto preceding single-wait NoOps on the same engine."""
    f = nc.m.functions[0]
    for bb in list(f.blocks):
        new, ctr = [], 0
        for inst in bb.instructions:
            si = inst.sync_info
            waits = list(si.on_wait) if (si and si.on_wait) else []
            if len(waits) > _MAXW:
                head, keep = waits[:-_MAXW], waits[-_MAXW:]
                for i in range(0, len(head), _MAXW):
                    nop = mybir.InstNoOp(
                        name=f"{inst.name}-wsplit{ctr}", engine=inst.engine,
                        ins=[], outs=[],
                        sync_info=mybir.SyncInfo(on_wait=head[i:i + _MAXW],
                                                 on_update=[]),
                    )
                    ctr += 1
                    new.append(nop)
                inst.sync_info = mybir.SyncInfo(
                    on_wait=keep,
                    on_update=list(si.on_update) if si.on_update else [])
            new.append(inst)
        bb.instructions = new


def build_nc():
    with _lean_drain():
        return _build_nc_inner()


def _build_nc_inner():
    nc = bass.Bass()
    a_d = nc.dram_tensor("a", [KT, NKT, NP], FP8, kind="ExternalInput")
    xe_d = nc.dram_tensor("xe", [KT, NKT, DIM], FP8, kind="ExternalInput")
    # pack8: [8, pt(T*NP) | wcomb(T*104)] in one transfer
    PK = T * NP + T * 104
    pk_d = nc.dram_tensor("pk", [8, PK], BF16, kind="ExternalInput")
    dt_d = nc.dram_tensor("dt", [24, T, NP], BF16, kind="ExternalInput")
    w1_d = nc.dram_tensor("w1", [40, DIM], BF16, kind="ExternalInput")
    rdeg_d = nc.dram_tensor("rdeg", [DIM, NP], BF16, kind="ExternalInput")
    out_d = nc.dram_tensor("out", [DIM, NP], BF16, kind="ExternalOutput")

    with TileContext(nc) as tc:
        with (
            tc.tile_pool(name="const", bufs=1) as cpool,
            tc.tile_pool(name="adma", bufs=NG) as apool,
            tc.tile_pool(name="pagg", bufs=2, space="PSUM") as pagg,
            tc.tile_pool(name="pp1", bufs=3, space="PSUM") as pp1,
            tc.tile_pool(name="pcm", bufs=2, space="PSUM") as pcm,
            tc.tile_pool(name="pwu", bufs=1, space="PSUM") as pwu,
        ):
            xe_t = cpool.tile([KT, NKT, DIM], FP8)
            rdeg_t = cpool.tile([DIM, NP], BF16)
            w1_t = cpool.tile([40, DIM], BF16)
            pk_t = cpool.tile([8, PK], BF16)
            dag_t = cpool.tile([40, T, NP], BF16)
            a_tiles = {}

            def a_dma(g):
                a_t = apool.tile([KT, KG, NP], FP8, tag="a", name=f"a{g}")
                nc.sync.dma_start(
                    out=a_t, in_=a_d[:, g * KG:(g + 1) * KG, :])
                a_tiles[g] = a_t

            # single SP ring in consumption order; small tensors slotted
            # early enough that their completion sems never gate the chain
            # ONE ring, adjacency stream uninterrupted; small tensors at
            # the tail ordered by first use (their sems land just before
            # the consumers need them). Any transfer interleaved into the
            # a-stream costs ~2-3us per chunk (measured v7/v9).
            nc.sync.dma_start(out=xe_t[:, 0:NKT // 2, :],
                              in_=xe_d[:, 0:NKT // 2, :])
            for g in range(NG // 2):
                a_dma(g)
            nc.sync.dma_start(out=xe_t[:, NKT // 2:NKT, :],
                              in_=xe_d[:, NKT // 2:NKT, :])
            for g in range(NG // 2, NG):
                a_dma(g)
            nc.sync.dma_start(out=rdeg_t, in_=rdeg_d[:, :])
            nc.sync.dma_start(out=w1_t, in_=w1_d[:, :])
            nc.sync.dma_start(out=dag_t[8:32, :, :], in_=dt_d[:, :, :])
            nc.sync.dma_start(out=pk_t, in_=pk_d[:, :])

            # views into the 8-partition pack
            ptv = pk_t[:, 0:T * NP]
            wcomb_t = pk_t[:, T * NP:PK]

            # ---- SBUF state ----
            wu_l = cpool.tile([KT, 8], BF16)
            wu_r = cpool.tile([KT, 128], BF16)
            nc.vector.memset(wu_l, 0.0)
            nc.vector.memset(wu_r, 0.0)
            nc.vector.memset(dag_t[32:40, 0, :], 0.0)
            aggsb = cpool.tile([DIM, 2, NH], BF16)
            h2_t = cpool.tile([8, T, NP], BF16)
            outt_t = cpool.tile([DIM, NP], BF16)

            # ---- PE warmup (HAM at full clock before chunk 0 lands) ----
            wu_p = pwu.tile([8, 128], F32, tag="wu")
            for i in range(NWU):
                nc.tensor.matmul(wu_p, wu_l, wu_r, start=True, stop=True)

            # ---- phase 1: agg^T[96, NH] per half, halves interleaved ----
            aggp = [pagg.tile([DIM, NH], F32, tag="aggp", name=f"aggp{h}")
                    for h in range(2)]

            def phase1_chunk(g):
                a_t = a_tiles[g]
                if g < NG - 1:
                    for j in range(KG):
                        k = g * KG + j
                        for h in range(2):
                            nc.tensor.matmul(
                                aggp[h], xe_t[:, k, :],
                                a_t[:, j, h * NH:(h + 1) * NH],
                                start=(k == 0), stop=False,
                                skip_group_check=True)
                else:
                    # final chunk: finish half 0 first so its transition
                    # and scatter start while half 1 is still contracting
                    for h in range(2):
                        for j in range(KG):
                            k = g * KG + j
                            nc.tensor.matmul(
                                aggp[h], xe_t[:, k, :],
                                a_t[:, j, h * NH:(h + 1) * NH],
                                start=False, stop=(k == NKT - 1),
                                skip_group_check=True)

            # transition: aggsb = aggp * (1/deg); then scatter slabs into
            # dag rows 16:24 via SBUF->SBUF DMAs issued from Sync (idle)
            def transition(h):
                cs = slice(h * NH, (h + 1) * NH)
                nc.vector.tensor_mul(aggsb[:, h, :], aggp[h], rdeg_t[:, cs])

            def scatter(h, t):
                cs = slice(h * NH, (h + 1) * NH)
                nc.sync.dma_start(out=dag_t[0:8, t, cs],
                                  in_=aggsb[t * 8:(t + 1) * 8, h, :])

            # ---- phase 2: sequential t-chain, one p1 matmul per step ----
            pcombs = [pcm.tile([104, NH], F32, tag="pcm", name=f"pcomb{h}")
                      for h in range(2)]

            def chain_step(h, t):
                cs = slice(h * NH, (h + 1) * NH)
                r8 = slice(t * 8, t * 8 + 8)
                pcomb = pcombs[h]
                p1 = pp1.tile([8, NH], F32, tag="p1", name=f"p1_{h}_{t}")
                nc.tensor.matmul(wu_p, wu_l, wu_r, start=True, stop=True)
                nc.tensor.matmul(p1, w1_t[:, r8], dag_t[:, t, cs],
                                 start=True, stop=True)
                nc.vector.scalar_tensor_tensor(
                    h2_t[:, t, cs], p1, 0.0,
                    ptv[:, t * NP + h * NH:t * NP + (h + 1) * NH],
                    op0=mybir.AluOpType.max, op1=mybir.AluOpType.add)
                nc.tensor.matmul(pcomb,
                                 wcomb_t[:, t * 104:(t + 1) * 104],
                                 h2_t[:, t, cs],
                                 start=(t == 0), stop=(t == T - 1),
                                 skip_group_check=True)
                if t < T - 1:
                    nc.scalar.activation(
                        dag_t[32:40, t + 1, cs], pcomb[DIM:104, :],
                        mybir.ActivationFunctionType.Relu)

            def final(h):
                cs = slice(h * NH, (h + 1) * NH)
                nc.scalar.activation(outt_t[:, cs], pcombs[h][0:DIM, :],
                                     mybir.ActivationFunctionType.Relu)
                nc.sync.dma_start(out=out_d[:, cs], in_=outt_t[:, cs])

            for g in range(NG):
                phase1_chunk(g)

            def direct_slab(h, t):
                cs = slice(h * NH, (h + 1) * NH)
                r8 = slice(t * 8, (t + 1) * 8)
                nc.vector.tensor_mul(dag_t[0:8, t, cs], aggp[h][r8, :],
                                     rdeg_t[r8, cs])

            direct_slab(0, 0)
            transition(0)
            direct_slab(1, 0)
            transition(1)
            for t in range(1, T):
                scatter(0, t)
                scatter(1, t)
            for i in range(12):
                nc.tensor.matmul(wu_p, wu_l, wu_r, start=True, stop=True)
            for t in range(T):
                chain_step(0, t)
                chain_step(1, t)
            final(0)
            final(1)

    split_multi_waits(nc)
    return nc


def prep_in_maps(adj, data, pos, his_W, cur_W, his_weight, cur_weight,
                 final_weight):
    adj = np.asarray(adj, dtype=np.float32)
    data = np.asarray(data, dtype=np.float32)
    pos = np.asarray(pos, dtype=np.float32)
    his_W = np.asarray(his_W, dtype=np.float32)
    cur_W = np.asarray(cur_W, dtype=np.float32)
    his_weight = np.asarray(his_weight, dtype=np.float32)
    cur_weight = np.asarray(cur_weight, dtype=np.float32)
    final_weight = np.asarray(final_weight, dtype=np.float32)

    deg = adj.sum(axis=1)
    rdeg_full = 1.0 / np.maximum(deg, 1.0)

    X = np.ascontiguousarray(data.transpose(1, 0, 2).reshape(N, DIM))
    Xe = np.zeros((NK, DIM), np.float32)
    Xe[:N, :] = X
    xe_h = np.ascontiguousarray(
        Xe.reshape(NKT, KT, DIM).transpose(1, 0, 2)).astype(FP8_NP)

    adjT = np.ascontiguousarray(adj.T).astype(FP8_NP)

    # w1 [40, 96]: per-t lhsT over dag rows [agg(0:8); raw(8:16);
    # zeros(16:32); prev(32:40)] (32-aligned prev for the ACT writes)
    w1 = np.zeros((40, DIM), np.float32)
    for t in range(T):
        w1[0:7, t * 8:t * 8 + 7] = his_W[t][:, 7:14].T
        w1[7, t * 8 + 7] = cur_W[t][0, 1]
        w1[8:15, t * 8:t * 8 + 7] = his_W[t][:, 0:7].T
        w1[15, t * 8 + 7] = cur_W[t][0, 0]
        w1[32:39, t * 8:t * 8 + 7] = his_W[t][:, 21:28].T
        w1[39, t * 8 + 7] = cur_W[t][0, 3]
    w2 = np.zeros((8, DIM), np.float32)
    for tp in range(T):
        w2[0:7, tp * 8:tp * 8 + 7] = his_weight[:, 7 * tp:7 * tp + 7].T
        w2[7, tp * 8 + 7] = cur_weight[0, tp]
    f_ref = np.array([7 * t + d if d < 7 else 84 + t
                      for t in range(T) for d in range(8)])
    wf96 = final_weight[:, f_ref].T
    wf = np.ascontiguousarray(
        wf96.reshape(T, 8, DIM).transpose(1, 0, 2).reshape(8, T * DIM))
    wcomb = np.zeros((8, T, 104), np.float32)
    for t in range(T):
        wcomb[:, t, 0:DIM] = wf[:, t * DIM:(t + 1) * DIM]
        wcomb[:, t, DIM:104] = w2[:, t * 8:(t + 1) * 8]
    wcomb = np.ascontiguousarray(wcomb.reshape(8, T * 104))

    in_maps = []
    for c in range(NCORES):
        c0 = c * NPC
        ac = np.zeros((NK, NP), FP8_NP)
        ac[:N, :NPC] = adjT[:, c0:c0 + NPC]
        ah = np.ascontiguousarray(
            ac.reshape(NKT, KT, NP).transpose(1, 0, 2))
        # rows 0:8 = data; rows 8:24 zero-fill dag rows 16:32 (the
        # contraction dead-zone) so no SBUF garbage meets the matmul
        dtc = np.zeros((24, T, NP), np.float32)
        dtc[0:8, :, :NPC] = data[:, c0:c0 + NPC, :].transpose(2, 0, 1)
        ptc = np.zeros((8, T, NP), np.float32)
        ptc[:, :, :NPC] = pos[:, c0:c0 + NPC, :].transpose(2, 0, 1)
        rdeg = np.zeros((DIM, NP), np.float32)
        rdeg[:, :NPC] = rdeg_full[c0:c0 + NPC][None, :]
        pk = np.concatenate([ptc.reshape(8, T * NP), wcomb], axis=1)
        in_maps.append({
            "a": ah, "xe": xe_h, "pk": pk.astype(BF16_NP),
            "dt": dtc.astype(BF16_NP),
            "w1": w1.astype(BF16_NP), "rdeg": rdeg.astype(BF16_NP),
        })
    return in_maps


def assemble(results):
    out = np.empty((N, DIM), np.float32)
    for c in range(NCORES):
        out[c * NPC:(c + 1) * NPC, :] = \
            results[c]["out"][:, :NPC].T.astype(np.float32)
    return out


_NC_CACHE = None


def get_nc():
    global _NC_CACHE
    if _NC_CACHE is None:
        _NC_CACHE = build_nc()
    return _NC_CACHE


def run_spmd(in_maps, **kwargs):
    nc = get_nc()
    return bass_utils.run_bass_kernel_spmd(
        nc, in_maps, list(range(NCORES)), **kwargs)


def kernel(**inputs):
    in_maps = prep_in_maps(**inputs)
    res = run_spmd(in_maps)
    return assemble(res.results)
